# revision 1
# baseline (speedup 1.0000x reference)
"""Trainium2 Bass kernel for CenterAlignment (segment-reduce + EMA + normalize + loss).

Contract: kernel(**inputs) takes FULL unsharded numpy inputs
  x:          [65536, 1024] f32
  center_img: [1000, 1024]  f32
  center_skt: [1000, 1024]  f32
  l:          [32768]       int64
and returns the full scalar loss (f32, shape ()).

Strategy (8 NeuronCores, SPMD):
  - Data-parallel shard of x / labels over the sample axis. Crop pairs
    (sample i and i+32768 share label l[i]) are pre-added on-chip, halving
    tensor-engine work.
  - Per-class partial sums via one-hot matmul: for each 128-sample tile,
    a [128, 1024(padded classes)] bf16 one-hot is built on the vector
    engine (fp16 iota vs label), then onehot^T @ xsum accumulates in PSUM.
  - Features are processed in 4 quarter passes of 256 columns (PSUM bank
    budget); pass 0 carries an extra constant-2.0 column so per-class
    counts fall out of the same matmuls.
  - x is loaded in batches of 8 sample-tiles per dma_start (3-D access
    patterns) to amortize per-DMA issue/semaphore overhead.
  - Each quarter's [1024, 256] partial is ReduceScatter'd across the 8
    cores as soon as its pass finishes (overlaps with later passes); each
    core ends up owning the global sums for its 128 classes.
  - The EMA + L2-normalize + masked alignment loss tail runs per-core on
    its 128-class slice; a tiny AllReduce combines [loss_sum, n_present]
    and every core computes the final scalar.
"""

import sys

for _p in ("/opt/trn_rl_repo",):
    if _p not in sys.path:
        sys.path.insert(0, _p)

import numpy as np

from concourse import bacc, bass, tile
from concourse import mybir
from concourse import bass_utils

f32 = mybir.dt.float32
f16 = mybir.dt.float16
bf16 = mybir.dt.bfloat16
i32 = mybir.dt.int32

N_CORES = 8
B = 32768              # labels per batch
NUM_CROPS = 2
FEA = 1024             # feature dim
C_PAD = 1024           # classes padded 1000 -> 1024 (8 chunks of 128)
N_CLASSES = 1000
Q = 256                # feature quarter width
N_CHUNKS = C_PAD // 128
MOMENTUM = 0.9


def build_program(rows_per_core: int = B * NUM_CROPS // N_CORES // NUM_CROPS,
                  _tail: str = "full"):
    """Build the SPMD Bass program (same graph on all 8 cores).

    rows_per_core: number of crop-PAIRS this core owns (default 4096).
    """
    assert rows_per_core % 128 == 0
    n_tiles = rows_per_core // 128
    batch = min(8, n_tiles)          # sample-tiles per x DMA
    assert n_tiles % batch == 0
    n_batches = n_tiles // batch

    nc = bacc.Bacc(
        "TRN2",
        target_bir_lowering=False,
        debug=False,
        enable_asserts=False,
        num_devices=N_CORES,
    )

    x01_d = nc.dram_tensor(
        "x01", [n_batches, NUM_CROPS, 128 * batch, FEA], f32,
        kind="ExternalInput",
    )
    lab_d = nc.dram_tensor("labels", [rows_per_core], i32, kind="ExternalInput")
    ci_d = nc.dram_tensor("ci", [128, FEA], f32, kind="ExternalInput")
    cs_d = nc.dram_tensor("cs", [128, FEA], f32, kind="ExternalInput")
    loss_d = nc.dram_tensor("loss", [1], f32, kind="ExternalOutput")

    groups = [list(range(N_CORES))]

    with tile.TileContext(nc) as tc:
        with (
            tc.tile_pool(name="const", bufs=1) as const_pool,
            tc.tile_pool(name="oh", bufs=1) as oh_pool,
            tc.tile_pool(name="x01p", bufs=2) as x01_pool,
            tc.tile_pool(name="xsp", bufs=2) as xs_pool,
            tc.tile_pool(name="qst", bufs=2) as qst_pool,
            tc.tile_pool(name="psum", bufs=1, space="PSUM") as psum_pool,
            tc.tile_pool(name="dram", bufs=1, space="DRAM") as dram_pool,
        ):
            # ---- constants / persistent tiles ----
            iota_t = const_pool.tile([128, C_PAD], f16, tag="iota")
            nc.gpsimd.iota(
                iota_t[:],
                pattern=[[1, C_PAD]],
                base=0,
                channel_multiplier=0,
                allow_small_or_imprecise_dtypes=True,
            )
            ones_t = const_pool.tile([128, 1], f32, tag="ones")
            nc.vector.memset(ones_t[:], 1.0)

            # labels: one strided DMA + one int->float convert
            lab_sb = const_pool.tile([128, n_tiles], i32, tag="lab32")
            nc.gpsimd.dma_start(
                lab_sb[:], lab_d[:].rearrange("(t p) -> p t", p=128)
            )
            labf = const_pool.tile([128, n_tiles], f32, tag="labf")
            nc.vector.tensor_copy(labf[:], lab_sb[:])

            ci_sb = const_pool.tile([128, FEA], f32, tag="ci")
            nc.gpsimd.dma_start(ci_sb[:], ci_d[:, :])
            cs_sb = const_pool.tile([128, FEA], f32, tag="cs")
            nc.gpsimd.dma_start(cs_sb[:], cs_d[:, :])

            # DRAM bounce buffers
            qbounce = [
                dram_pool.tile([C_PAD, Q + 1 if q == 0 else Q], bf16,
                               tag=f"qb{q}", name=f"qb{q}")
                for q in range(4)
            ]
            rs_q = [
                dram_pool.tile([C_PAD // N_CORES, Q + 1 if q == 0 else Q], bf16,
                               tag=f"rs{q}", name=f"rs{q}")
                for q in range(4)
            ]
            ag_in = dram_pool.tile([1, 2], f32, tag="agi")
            ag_out = dram_pool.tile([N_CORES, 2], f32, tag="ago")

            ohs = [None] * n_tiles

            # ---- 4 feature-quarter passes ----
            for q in range(4):
                w = Q + 1 if q == 0 else Q  # pass 0 carries the counts column
                accs = [
                    psum_pool.tile([128, w], f32, tag=f"acc{c}", name=f"acc{c}")
                    for c in range(N_CHUNKS)
                ]
                for b in range(n_batches):
                    cols = bass.ts(q, Q)
                    x01b = x01_pool.tile([128, NUM_CROPS * batch, Q], f32,
                                         tag="x01b")
                    nc.sync.dma_start(
                        x01b[:],
                        x01_d[b, :, :, cols].rearrange(
                            "k (j p) c -> p (k j) c", p=128
                        ),
                    )
                    xsb = xs_pool.tile([128, batch, w], bf16, tag="xsb")
                    nc.vector.tensor_tensor(
                        xsb[:, :, 0:Q],
                        x01b[:, 0:batch, :],
                        x01b[:, batch : 2 * batch, :],
                        op=mybir.AluOpType.add,
                    )
                    if q == 0:
                        nc.vector.memset(xsb[:, :, Q : Q + 1], 2.0)
                        # build this batch's one-hot tiles (resident afterwards)
                        for j in range(batch):
                            t = b * batch + j
                            oh_t = oh_pool.tile(
                                [128, C_PAD], bf16, tag=f"oh{t}", name=f"oh{t}"
                            )
                            nc.vector.tensor_scalar(
                                oh_t[:],
                                iota_t[:],
                                labf[:, t : t + 1],
                                None,
                                op0=mybir.AluOpType.is_equal,
                            )
                            ohs[t] = oh_t
                    for j in range(batch):
                        t = b * batch + j
                        for c in range(N_CHUNKS):
                            nc.tensor.matmul(
                                accs[c][:],
                                ohs[t][:, bass.ts(c, 128)],
                                xsb[:, j, :],
                                start=(t == 0),
                                stop=(t == n_tiles - 1),
                            )
                # drain accumulators: stage all chunks, one batched DMA out
                qstage = qst_pool.tile([128, N_CHUNKS, w], bf16, tag="qstage")
                for c in range(N_CHUNKS):
                    nc.scalar.copy(qstage[:, c, :], accs[c][:])
                nc.gpsimd.dma_start(
                    qbounce[q][:].rearrange("(c p) f -> p c f", p=128),
                    qstage[:],
                )
                nc.gpsimd.collective_compute(
                    "ReduceScatter",
                    mybir.AluOpType.add,
                    replica_groups=groups,
                    ins=[qbounce[q][:].opt()],
                    outs=[rs_q[q][:].opt()],
                )

            if _tail == "none":
                lsb0 = const_pool.tile([1, 1], f32, tag="lsb0")
                nc.vector.tensor_copy(lsb0[:], labf[0:1, 0:1])
                nc.gpsimd.dma_start(
                    loss_d[:].rearrange("(p o) -> p o", o=1), lsb0[:]
                )

            if _tail == "full":
                # ---- tail: EMA + normalize + masked loss on this core's classes ----
                msums = const_pool.tile([128, FEA], bf16, tag="msums")
                for q in range(4):
                    nc.gpsimd.dma_start(msums[:, bass.ts(q, Q)], rs_q[q][:, 0:Q])
                mcntb = const_pool.tile([128, 1], bf16, tag="mcntb")
                nc.gpsimd.dma_start(mcntb[:], rs_q[0][:, Q : Q + 1])
                mcnt = const_pool.tile([128, 1], f32, tag="mcnt")
                nc.vector.tensor_copy(mcnt[:], mcntb[:])

                cnt1 = const_pool.tile([128, 1], f32, tag="cnt1")
                nc.vector.tensor_scalar_max(cnt1[:], mcnt[:], 1.0)
                rec = const_pool.tile([128, 1], f32, tag="rec")
                nc.vector.reciprocal(rec[:], cnt1[:])
                pres = const_pool.tile([128, 1], f32, tag="pres")
                nc.vector.tensor_scalar_min(pres[:], mcnt[:], 1.0)

                # mean*(1-momentum) = sums * (1/count) * 0.1
                msc = const_pool.tile([128, FEA], f32, tag="tailA")
                nc.vector.tensor_scalar(
                    msc[:],
                    msums[:],
                    rec[:],
                    1.0 - MOMENTUM,
                    op0=mybir.AluOpType.mult,
                    op1=mybir.AluOpType.mult,
                )
                # upd = ci*momentum + mean*(1-momentum)
                upd = const_pool.tile([128, FEA], f32, tag="tailB")
                nc.vector.scalar_tensor_tensor(
                    upd[:],
                    in0=ci_sb[:],
                    scalar=MOMENTUM,
                    in1=msc[:],
                    op0=mybir.AluOpType.mult,
                    op1=mybir.AluOpType.add,
                )
                # ss = sum(upd*upd) per class row (Square + row-accumulate on ACT)
                tmp = const_pool.tile([128, FEA], f32, tag="tailC")
                ss = const_pool.tile([128, 1], f32, tag="ss")
                nc.scalar.activation(
                    tmp[:],
                    upd[:],
                    mybir.ActivationFunctionType.Square,
                    accum_out=ss[:],
                )
                ssg = const_pool.tile([128, 1], f32, tag="ssg")
                nc.vector.tensor_scalar_max(ssg[:], ss[:], 1e-30)
                ssr = const_pool.tile([128, 1], f32, tag="ssr")
                nc.vector.reciprocal(ssr[:], ssg[:])
                rnorm = const_pool.tile([128, 1], f32, tag="rnorm")
                nc.scalar.activation(
                    rnorm[:], ssr[:], mybir.ActivationFunctionType.Sqrt
                )
                newc = const_pool.tile([128, FEA], f32, tag="tailA")
                nc.vector.tensor_scalar(
                    newc[:], upd[:], rnorm[:], None, op0=mybir.AluOpType.mult
                )
                diff = const_pool.tile([128, FEA], f32, tag="tailB")
                nc.vector.tensor_tensor(
                    diff[:], newc[:], cs_sb[:], op=mybir.AluOpType.subtract
                )
                tmp2 = const_pool.tile([128, FEA], f32, tag="tailC")
                pc = const_pool.tile([128, 1], f32, tag="pc")
                nc.scalar.activation(
                    tmp2[:],
                    diff[:],
                    mybir.ActivationFunctionType.Square,
                    accum_out=pc[:],
                )
                stack = const_pool.tile([128, 2], f32, tag="stack")
                nc.vector.tensor_tensor(
                    stack[:, 0:1], pc[:], pres[:], op=mybir.AluOpType.mult
                )
                nc.vector.tensor_copy(stack[:, 1:2], pres[:])

                red_ps = psum_pool.tile([1, 2], f32, tag="acc0")
                nc.tensor.matmul(
                    red_ps[:], ones_t[:], stack[:], start=True, stop=True
                )
                red_sb = const_pool.tile([1, 2], f32, tag="redsb")
                nc.scalar.copy(red_sb[:], red_ps[:])
                nc.gpsimd.dma_start(ag_in[:, :], red_sb[:])
                nc.gpsimd.collective_compute(
                    "AllGather",
                    mybir.AluOpType.bypass,
                    replica_groups=groups,
                    ins=[ag_in[:].opt()],
                    outs=[ag_out[:].opt()],
                )
                ag_sb = const_pool.tile([1, N_CORES * 2], f32, tag="agsb")
                nc.gpsimd.dma_start(
                    ag_sb[:],
                    ag_out[:, :].rearrange("r c -> (r c)").rearrange(
                        "(p f) -> p f", p=1
                    ),
                )
                # tree-add the 8 [loss, n] pairs (r-major layout keeps pairs aligned)
                f8 = const_pool.tile([1, 8], f32, tag="f8")
                nc.vector.tensor_tensor(
                    f8[:], ag_sb[:, 0:8], ag_sb[:, 8:16], op=mybir.AluOpType.add
                )
                f4 = const_pool.tile([1, 4], f32, tag="f4")
                nc.vector.tensor_tensor(
                    f4[:], f8[:, 0:4], f8[:, 4:8], op=mybir.AluOpType.add
                )
                fin = const_pool.tile([1, 2], f32, tag="fin")
                nc.vector.tensor_tensor(
                    fin[:], f4[:, 0:2], f4[:, 2:4], op=mybir.AluOpType.add
                )
                finv = const_pool.tile([1, 1], f32, tag="finv")
                nc.vector.reciprocal(finv[:], fin[:, 1:2])
                lsb = const_pool.tile([1, 1], f32, tag="lsb")
                nc.vector.tensor_tensor(
                    lsb[:], fin[:, 0:1], finv[:], op=mybir.AluOpType.mult
                )
                nc.gpsimd.dma_start(
                    loss_d[:].rearrange("(p o) -> p o", o=1), lsb[:]
                )

    nc.compile()
    return nc


def build_program_fp8(rows_per_core: int = B * NUM_CROPS // N_CORES // NUM_CROPS):
    """fp8e4m3 DoubleRow variant: 2 feature passes of 512, counts via DVE
    one-hot accumulation + end matmul. ~2x fewer, faster matmuls."""
    assert rows_per_core % 256 == 0
    n_tiles = rows_per_core // 128
    batch = min(8, n_tiles)          # sample-tiles per x DMA
    assert n_tiles % batch == 0 and batch % 2 == 0
    n_batches = n_tiles // batch
    W = 512
    fp8 = mybir.dt.float8e4

    nc = bacc.Bacc(
        "TRN2",
        target_bir_lowering=False,
        debug=False,
        enable_asserts=False,
        num_devices=N_CORES,
    )

    x01_d = nc.dram_tensor(
        "x01", [n_batches, NUM_CROPS, 128 * batch, FEA], f32,
        kind="ExternalInput",
    )
    lab_d = nc.dram_tensor("labels", [rows_per_core], i32, kind="ExternalInput")
    ci_d = nc.dram_tensor("ci", [128, FEA], f32, kind="ExternalInput")
    cs_d = nc.dram_tensor("cs", [128, FEA], f32, kind="ExternalInput")
    loss_d = nc.dram_tensor("loss", [1], f32, kind="ExternalOutput")

    groups = [list(range(N_CORES))]

    with tile.TileContext(nc) as tc:
        with (
            tc.tile_pool(name="const", bufs=1) as const_pool,
            tc.tile_pool(name="oh", bufs=1) as oh_pool,
            tc.tile_pool(name="x01p", bufs=2) as x01_pool,
            tc.tile_pool(name="xsp", bufs=2) as xs_pool,
            tc.tile_pool(name="qst", bufs=2) as qst_pool,
            tc.tile_pool(name="psum", bufs=1, space="PSUM") as psum_pool,
            tc.tile_pool(name="dram", bufs=1, space="DRAM") as dram_pool,
        ):
            iota_t = const_pool.tile([128, C_PAD], f16, tag="iota")
            nc.gpsimd.iota(
                iota_t[:],
                pattern=[[1, C_PAD]],
                base=0,
                channel_multiplier=0,
                allow_small_or_imprecise_dtypes=True,
            )
            twos_t = const_pool.tile([128, 1], f32, tag="twos")
            nc.vector.memset(twos_t[:], 2.0)
            oh_acc = const_pool.tile([128, C_PAD], f32, tag="ohacc")
            nc.vector.memset(oh_acc[:], 0.0)

            lab_sb = const_pool.tile([128, n_tiles], i32, tag="lab32")
            nc.gpsimd.dma_start(
                lab_sb[:], lab_d[:].rearrange("(t p) -> p t", p=128)
            )
            labf = const_pool.tile([128, n_tiles], f32, tag="labf")
            nc.vector.tensor_copy(labf[:], lab_sb[:])

            ci_sb = const_pool.tile([128, FEA], f32, tag="ci")
            nc.gpsimd.dma_start(ci_sb[:], ci_d[:, :])
            cs_sb = const_pool.tile([128, FEA], f32, tag="cs")
            nc.gpsimd.dma_start(cs_sb[:], cs_d[:, :])

            qbounce = [
                dram_pool.tile([C_PAD, W + q], bf16, tag=f"qb{q}", name=f"qb{q}")
                for q in range(2)
            ]
            rs_q = [
                dram_pool.tile([C_PAD // N_CORES, W + q], bf16, tag=f"rs{q}",
                               name=f"rs{q}")
                for q in range(2)
            ]
            ag_in = dram_pool.tile([1, 2], f32, tag="agi")
            ag_out = dram_pool.tile([N_CORES, 2], f32, tag="ago")

            ohps = [None] * (n_tiles // 2)

            for q in range(2):
                accs = [
                    psum_pool.tile([128, W], f32, tag=f"acc{c}", name=f"acc{c}")
                    for c in range(N_CHUNKS)
                ]
                for b in range(n_batches):
                    cols = bass.ts(q, W)
                    x01b = x01_pool.tile([128, NUM_CROPS * batch, W], f32,
                                         tag="x01b")
                    nc.sync.dma_start(
                        x01b[:, 0:batch, :],
                        x01_d[b, 0, :, cols].rearrange(
                            "(j p) c -> p j c", p=128
                        ),
                    )
                    nc.scalar.dma_start(
                        x01b[:, batch : 2 * batch, :],
                        x01_d[b, 1, :, cols].rearrange(
                            "(j p) c -> p j c", p=128
                        ),
                    )
                    xsb = xs_pool.tile([128, batch, W], fp8, tag="xsb")
                    nc.vector.tensor_tensor(
                        xsb[:],
                        x01b[:, 0:batch, :],
                        x01b[:, batch : 2 * batch, :],
                        op=mybir.AluOpType.add,
                    )
                    if q == 0:
                        for v in range(batch // 2):
                            u = b * (batch // 2) + v
                            ohp = oh_pool.tile(
                                [128, 2, C_PAD], fp8, tag=f"ohp{u}",
                                name=f"ohp{u}",
                            )
                            for jj in range(2):
                                t = b * batch + 2 * v + jj
                                nc.vector.tensor_scalar(
                                    ohp[:, jj, :],
                                    iota_t[:],
                                    labf[:, t : t + 1],
                                    None,
                                    op0=mybir.AluOpType.is_equal,
                                )
                            ohps[u] = ohp
                    else:
                        # counts accumulation rides pass 1's idle DVE slots
                        for v in range(batch // 2):
                            u = b * (batch // 2) + v
                            for jj in range(2):
                                nc.vector.tensor_tensor(
                                    oh_acc[:], oh_acc[:], ohps[u][:, jj, :],
                                    op=mybir.AluOpType.add,
                                )
                    for v in range(batch // 2):
                        u = b * (batch // 2) + v
                        for c in range(N_CHUNKS):
                            nc.tensor.matmul(
                                accs[c][:],
                                ohps[u][:, :, bass.ts(c, 128)],
                                xsb[:, 2 * v : 2 * v + 2, :],
                                perf_mode=mybir.MatmulPerfMode.DoubleRow,
                                start=(u == 0),
                                stop=(u == n_tiles // 2 - 1),
                            )
                qstage = qst_pool.tile([128, N_CHUNKS, W], bf16, tag="qstage")
                for c in range(N_CHUNKS):
                    nc.scalar.copy(qstage[:, c, :], accs[c][:])
                nc.gpsimd.dma_start(
                    qbounce[q][:, 0:W].rearrange("(c p) f -> p c f", p=128),
                    qstage[:],
                )
                if q == 1:
                    # counts = 2 * colsum(oh_acc) via two small f32 matmuls
                    cnt_sb = const_pool.tile([1, C_PAD], bf16, tag="cntsb")
                    for h in range(2):
                        cnt_ps = psum_pool.tile(
                            [1, W], f32, tag=f"acc{h}", name=f"cnt{h}"
                        )
                        nc.tensor.matmul(
                            cnt_ps[:], twos_t[:],
                            oh_acc[:, bass.ts(h, W)],
                            start=True, stop=True,
                        )
                        nc.scalar.copy(cnt_sb[:, bass.ts(h, W)], cnt_ps[:])
                    nc.gpsimd.dma_start(
                        qbounce[1][:, W : W + 1],
                        cnt_sb[:].rearrange("o (c oo) -> (o c) oo", oo=1),
                    )
                nc.gpsimd.collective_compute(
                    "ReduceScatter",
                    mybir.AluOpType.add,
                    replica_groups=groups,
                    ins=[qbounce[q][:].opt()],
                    outs=[rs_q[q][:].opt()],
                )

            # ---- tail ----
            msums = const_pool.tile([128, FEA], bf16, tag="msums")
            for q in range(2):
                nc.gpsimd.dma_start(msums[:, bass.ts(q, W)], rs_q[q][:, 0:W])
            mcntb = const_pool.tile([128, 1], bf16, tag="mcntb")
            nc.gpsimd.dma_start(mcntb[:], rs_q[1][:, W : W + 1])
            mcnt = const_pool.tile([128, 1], f32, tag="mcnt")
            nc.vector.tensor_copy(mcnt[:], mcntb[:])

            cnt1 = const_pool.tile([128, 1], f32, tag="cnt1")
            nc.vector.tensor_scalar_max(cnt1[:], mcnt[:], 1.0)
            rec = const_pool.tile([128, 1], f32, tag="rec")
            nc.vector.reciprocal(rec[:], cnt1[:])
            pres = const_pool.tile([128, 1], f32, tag="pres")
            nc.vector.tensor_scalar_min(pres[:], mcnt[:], 1.0)

            msc = const_pool.tile([128, FEA], f32, tag="tailA")
            nc.vector.tensor_scalar(
                msc[:],
                msums[:],
                rec[:],
                1.0 - MOMENTUM,
                op0=mybir.AluOpType.mult,
                op1=mybir.AluOpType.mult,
            )
            upd = const_pool.tile([128, FEA], f32, tag="tailB")
            nc.vector.scalar_tensor_tensor(
                upd[:],
                in0=ci_sb[:],
                scalar=MOMENTUM,
                in1=msc[:],
                op0=mybir.AluOpType.mult,
                op1=mybir.AluOpType.add,
            )
            tmp = const_pool.tile([128, FEA], f32, tag="tailC")
            ss = const_pool.tile([128, 1], f32, tag="ss")
            nc.scalar.activation(
                tmp[:],
                upd[:],
                mybir.ActivationFunctionType.Square,
                accum_out=ss[:],
            )
            ssg = const_pool.tile([128, 1], f32, tag="ssg")
            nc.vector.tensor_scalar_max(ssg[:], ss[:], 1e-30)
            ssr = const_pool.tile([128, 1], f32, tag="ssr")
            nc.vector.reciprocal(ssr[:], ssg[:])
            rnorm = const_pool.tile([128, 1], f32, tag="rnorm")
            nc.scalar.activation(
                rnorm[:], ssr[:], mybir.ActivationFunctionType.Sqrt
            )
            newc = const_pool.tile([128, FEA], f32, tag="tailA")
            nc.vector.tensor_scalar(
                newc[:], upd[:], rnorm[:], None, op0=mybir.AluOpType.mult
            )
            diff = const_pool.tile([128, FEA], f32, tag="tailB")
            nc.vector.tensor_tensor(
                diff[:], newc[:], cs_sb[:], op=mybir.AluOpType.subtract
            )
            tmp2 = const_pool.tile([128, FEA], f32, tag="tailC")
            pc = const_pool.tile([128, 1], f32, tag="pc")
            nc.scalar.activation(
                tmp2[:],
                diff[:],
                mybir.ActivationFunctionType.Square,
                accum_out=pc[:],
            )
            stack = const_pool.tile([128, 2], f32, tag="stack")
            nc.vector.tensor_tensor(
                stack[:, 0:1], pc[:], pres[:], op=mybir.AluOpType.mult
            )
            nc.vector.tensor_copy(stack[:, 1:2], pres[:])

            red_ps = psum_pool.tile([1, 2], f32, tag="acc2")
            ones_t = const_pool.tile([128, 1], f32, tag="ones")
            nc.vector.memset(ones_t[:], 1.0)
            nc.tensor.matmul(
                red_ps[:], ones_t[:], stack[:], start=True, stop=True
            )
            red_sb = const_pool.tile([1, 2], f32, tag="redsb")
            nc.scalar.copy(red_sb[:], red_ps[:])
            nc.gpsimd.dma_start(ag_in[:, :], red_sb[:])
            nc.gpsimd.collective_compute(
                "AllGather",
                mybir.AluOpType.bypass,
                replica_groups=groups,
                ins=[ag_in[:].opt()],
                outs=[ag_out[:].opt()],
            )
            ag_sb = const_pool.tile([1, N_CORES * 2], f32, tag="agsb")
            nc.gpsimd.dma_start(
                ag_sb[:],
                ag_out[:, :].rearrange("r c -> (r c)").rearrange(
                    "(p f) -> p f", p=1
                ),
            )
            f8 = const_pool.tile([1, 8], f32, tag="f8")
            nc.vector.tensor_tensor(
                f8[:], ag_sb[:, 0:8], ag_sb[:, 8:16], op=mybir.AluOpType.add
            )
            f4 = const_pool.tile([1, 4], f32, tag="f4")
            nc.vector.tensor_tensor(
                f4[:], f8[:, 0:4], f8[:, 4:8], op=mybir.AluOpType.add
            )
            fin = const_pool.tile([1, 2], f32, tag="fin")
            nc.vector.tensor_tensor(
                fin[:], f4[:, 0:2], f4[:, 2:4], op=mybir.AluOpType.add
            )
            finv = const_pool.tile([1, 1], f32, tag="finv")
            nc.vector.reciprocal(finv[:], fin[:, 1:2])
            lsb = const_pool.tile([1, 1], f32, tag="lsb")
            nc.vector.tensor_tensor(
                lsb[:], fin[:, 0:1], finv[:], op=mybir.AluOpType.mult
            )
            nc.gpsimd.dma_start(
                loss_d[:].rearrange("(p o) -> p o", o=1), lsb[:]
            )

    nc.compile()
    return nc


def make_in_maps(x, center_img, center_skt, l, rows_per_core=None, batch=8):
    """Shard full inputs into per-core input maps."""
    n = x.shape[0] // NUM_CROPS
    if rows_per_core is None:
        rows_per_core = n // N_CORES
    x = np.ascontiguousarray(x, dtype=np.float32)
    l = np.ascontiguousarray(l).astype(np.int32)
    n_tiles = rows_per_core // 128
    batch = min(batch, n_tiles)
    n_batches = n_tiles // batch
    ci_pad = np.zeros((C_PAD, FEA), np.float32)
    ci_pad[: center_img.shape[0]] = center_img
    cs_pad = np.zeros((C_PAD, FEA), np.float32)
    cs_pad[: center_skt.shape[0]] = center_skt
    in_maps = []
    for k in range(N_CORES):
        r0 = k * rows_per_core
        r1 = r0 + rows_per_core
        in_maps.append(
            {
                "x01": np.ascontiguousarray(
                    np.stack([x[r0:r1], x[n + r0 : n + r1]])
                    .reshape(NUM_CROPS, n_batches, 128 * batch, FEA)
                    .swapaxes(0, 1)
                ),
                "labels": np.ascontiguousarray(l[r0:r1]),
                "ci": np.ascontiguousarray(ci_pad[k * 128 : (k + 1) * 128]),
                "cs": np.ascontiguousarray(cs_pad[k * 128 : (k + 1) * 128]),
            }
        )
    return in_maps


_CACHED_NC = None


def _get_nc():
    global _CACHED_NC
    if _CACHED_NC is None:
        _CACHED_NC = build_program()
    return _CACHED_NC


def kernel(x, center_img, center_skt, l):
    nc = _get_nc()
    in_maps = make_in_maps(x, center_img, center_skt, l)
    res = bass_utils.run_bass_kernel_spmd(nc, in_maps, core_ids=list(range(N_CORES)))
    loss = np.asarray(res.results[0]["loss"], dtype=np.float32)
    return loss.reshape(()).astype(np.float32)



# revision 15
# speedup vs baseline: 1.2652x; 1.2652x over previous
"""Trainium2 Bass kernel for CenterAlignment (segment-reduce + EMA + normalize + loss).

Contract: kernel(**inputs) takes FULL unsharded numpy inputs
  x:          [65536, 1024] f32
  center_img: [1000, 1024]  f32
  center_skt: [1000, 1024]  f32
  l:          [32768]       int64
and returns the full scalar loss (f32, shape ()).

Strategy (8 NeuronCores, SPMD):
  - Data-parallel shard of x / labels over the sample axis. Each core gets
    two contiguous row slices (crop0/crop1 views - no host copies); crop
    pairs share a label and are pre-added on-chip, halving matmul work.
  - Per-class partial sums via one-hot matmul: per 128-sample tile a
    [128, 1024(padded classes)] fp8 one-hot is built on the vector engine
    (f16 iota vs label). Tiles are processed in PAIRS with fp8 DoubleRow
    matmuls (2x tensor-engine throughput): [128,2,128]^T @ [128,2,258].
  - Features go in 4 quarter passes of 256 columns (PSUM bank budget);
    pass 0 carries two extra constant-2.0 columns (keeping DoubleRow
    output widths even - odd widths hard-fault the PE) so per-class counts
    fall out of the same matmuls. PSUM chunks drain as soon as their last
    matmul retires so the next pass starts without a bank stall.
  - Each quarter's [1024, 256(+2)] partial is ReduceScatter'd (bf16)
    across the 8 cores as soon as its pass finishes, overlapping later
    passes; each core ends up owning global sums for its 128 classes.
  - Tail per core on its 128 classes, split per feature quarter so only
    the last quarter's work is exposed after the final ReduceScatter:
    with S1=sum(upd^2), S12=sum((upd+cs)^2), S3=sum(cs^2) accumulated per
    quarter,  ||upd/||upd|| - cs||^2 = (1+S3) - (S12-S1-S3)/sqrt(S1).
  - Each core outputs [128, 2] = (masked per-class loss, present flag);
    the final 8-way sum + divide happens on host as part of unsharding
    (no device AllGather on the critical path).
"""

import sys

for _p in ("/opt/trn_rl_repo",):
    if _p not in sys.path:
        sys.path.insert(0, _p)

import numpy as np

from concourse import bacc, bass, tile
from concourse import mybir
from concourse import bass_utils

f32 = mybir.dt.float32
f16 = mybir.dt.float16
bf16 = mybir.dt.bfloat16
fp8 = mybir.dt.float8e4
i32 = mybir.dt.int32

N_CORES = 8
B = 32768              # labels per batch
NUM_CROPS = 2
FEA = 1024             # feature dim
C_PAD = 1024           # classes padded 1000 -> 1024 (8 chunks of 128)
N_CLASSES = 1000
Q = 256                # feature quarter width
N_CHUNKS = C_PAD // 128
MOMENTUM = 0.9
ROWS_PER_CORE = B // N_CORES          # 4096 crop-pair rows per core


def build_program(mm: str = "fp8", rows_per_core: int = ROWS_PER_CORE):
    """Build the SPMD Bass program (same graph on all 8 cores).

    mm: "fp8" (DoubleRow pair matmuls) or "bf16" (per-tile matmuls).
    """
    assert rows_per_core % 256 == 0
    n_tiles = rows_per_core // 128
    batch = min(8, n_tiles)          # sample-tiles per x DMA
    assert n_tiles % batch == 0 and batch % 2 == 0
    n_batches = n_tiles // batch
    n_pairs = n_tiles // 2
    last_u = (n_pairs if mm == "fp8" else n_tiles) - 1
    cntc = 2 if mm == "fp8" else 1   # counts columns (even width for DoubleRow)

    nc = bacc.Bacc(
        "TRN2",
        target_bir_lowering=False,
        debug=False,
        enable_asserts=False,
        num_devices=N_CORES,
    )

    x0_d = nc.dram_tensor("x0", [rows_per_core, FEA], f32, kind="ExternalInput")
    x1_d = nc.dram_tensor("x1", [rows_per_core, FEA], f32, kind="ExternalInput")
    lab_d = nc.dram_tensor("labels", [rows_per_core], i32, kind="ExternalInput")
    ci_d = nc.dram_tensor("ci", [128, FEA], f32, kind="ExternalInput")
    cs_d = nc.dram_tensor("cs", [128, FEA], f32, kind="ExternalInput")
    out_d = nc.dram_tensor("loss", [128, 2], f32, kind="ExternalOutput")

    # row r of this core's slice lives at partition r // n_tiles, tile r %
    # n_tiles (labels land contiguously per partition: one 128B chunk each)
    x0_r = x0_d[:, :].rearrange("(p t) c -> p t c", p=128)
    x1_r = x1_d[:, :].rearrange("(p t) c -> p t c", p=128)

    groups = [list(range(N_CORES))]
    mdt = fp8 if mm == "fp8" else bf16
    Sq = mybir.ActivationFunctionType.Square

    with tile.TileContext(nc) as tc:
        with (
            tc.tile_pool(name="const", bufs=1) as const_pool,
            tc.tile_pool(name="oh", bufs=1) as oh_pool,
            tc.tile_pool(name="x01p", bufs=3) as x01_pool,
            tc.tile_pool(name="xsp", bufs=2) as xs_pool,
            tc.tile_pool(name="qst", bufs=2) as qst_pool,
            tc.tile_pool(name="psum", bufs=1, space="PSUM") as psum_pool,
            tc.tile_pool(name="dram", bufs=1, space="DRAM") as dram_pool,
        ):
            # ---- constants / persistent tiles ----
            lab_sb = const_pool.tile([128, n_tiles], i32, tag="lab32")
            nc.gpsimd.dma_start(
                lab_sb[:], lab_d[:].rearrange("(p t) -> p t", p=128)
            )
            iota_t = const_pool.tile([128, C_PAD], f16, tag="iota")
            nc.gpsimd.iota(
                iota_t[:],
                pattern=[[1, C_PAD]],
                base=0,
                channel_multiplier=0,
                allow_small_or_imprecise_dtypes=True,
            )
            labf = const_pool.tile([128, n_tiles], f32, tag="labf")
            nc.vector.tensor_copy(labf[:], lab_sb[:])

            # pre-warm the ACT function tables used by the tail
            warm = const_pool.tile([1, 1], f32, tag="warm")
            warm2 = const_pool.tile([1, 1], f32, tag="warm2")
            nc.vector.memset(warm[:], 1.0)
            nc.scalar.activation(warm2[:], warm[:], Sq)
            nc.scalar.activation(
                warm2[:], warm[:], mybir.ActivationFunctionType.Sqrt
            )

            # DRAM bounce buffers
            qbounce = [
                dram_pool.tile([C_PAD, Q + cntc if q == 0 else Q], bf16,
                               tag=f"qb{q}", name=f"qb{q}")
                for q in range(4)
            ]
            rs_q = [
                dram_pool.tile([C_PAD // N_CORES, Q + cntc if q == 0 else Q], bf16,
                               tag=f"rs{q}", name=f"rs{q}")
                for q in range(4)
            ]

            ohs = [None] * (n_pairs if mm == "fp8" else n_tiles)
            msums = const_pool.tile([128, FEA], bf16, tag="msums")
            mcntb = const_pool.tile([128, 1], bf16, tag="mcntb")

            # ---- 4 feature-quarter passes ----
            qstages = [None] * 4

            def issue_rs(q):
                """Stage pass q's PSUM drain out to DRAM and ReduceScatter it.

                Called from pass q+1 after its first x loads are triggered, so
                the qbounce wait (on pass q's drains) never starves the DMA
                FIFO of x work, yet only ~1 batch of x sits ahead of it.
                """
                nc.sync.dma_start(
                    qbounce[q][:].rearrange("(c p) f -> p c f", p=128),
                    qstages[q][:],
                )
                nc.gpsimd.collective_compute(
                    "ReduceScatter",
                    mybir.AluOpType.add,
                    replica_groups=groups,
                    ins=[qbounce[q][:].opt()],
                    outs=[rs_q[q][:].opt()],
                )

            for q in range(4):
                w = Q + cntc if q == 0 else Q  # pass 0 carries the counts column(s)
                accs = [
                    psum_pool.tile([128, w], f32, tag=f"acc{c}", name=f"acc{c}")
                    for c in range(N_CHUNKS)
                ]
                qstage = qst_pool.tile([128, N_CHUNKS, w], bf16, tag="qstage")
                qstages[q] = qstage
                for b in range(n_batches):
                    cols = bass.ts(q, Q)
                    tsl = slice(b * batch, (b + 1) * batch)
                    x01b = x01_pool.tile([128, NUM_CROPS * batch, Q], f32,
                                         tag="x01b")
                    nc.sync.dma_start(x01b[:, 0:batch, :], x0_r[:, tsl, cols])
                    nc.sync.dma_start(
                        x01b[:, batch : 2 * batch, :], x1_r[:, tsl, cols]
                    )
                    if b == 1 and q >= 1:
                        issue_rs(q - 1)
                        if q == 3:
                            nc.scalar.dma_start(
                                msums[:, bass.ts(0, Q)], rs_q[0][:, 0:Q]
                            )
                            nc.scalar.dma_start(mcntb[:], rs_q[0][:, Q : Q + 1])
                    xsb = xs_pool.tile([128, batch, w], mdt, tag="xsb")
                    last_pass_tail = q == 3 and b == n_batches - 1
                    if last_pass_tail:
                        # split the final add so the last matmuls start sooner
                        nc.vector.tensor_tensor(
                            xsb[:, 0 : batch // 2, 0:Q],
                            x01b[:, 0 : batch // 2, :],
                            x01b[:, batch : batch + batch // 2, :],
                            op=mybir.AluOpType.add,
                        )
                        nc.vector.tensor_tensor(
                            xsb[:, batch // 2 : batch, 0:Q],
                            x01b[:, batch // 2 : batch, :],
                            x01b[:, batch + batch // 2 : 2 * batch, :],
                            op=mybir.AluOpType.add,
                        )
                    else:
                        nc.vector.tensor_tensor(
                            xsb[:, :, 0:Q],
                            x01b[:, 0:batch, :],
                            x01b[:, batch : 2 * batch, :],
                            op=mybir.AluOpType.add,
                        )
                    if q == 0:
                        nc.vector.memset(xsb[:, :, Q : Q + cntc], 2.0)
                        # build this batch's one-hot tiles (resident afterwards)
                        if mm == "fp8":
                            for v in range(batch // 2):
                                u = b * (batch // 2) + v
                                ohp = oh_pool.tile(
                                    [128, 2, C_PAD], fp8, tag=f"ohp{u}",
                                    name=f"ohp{u}",
                                )
                                for jj in range(2):
                                    t = b * batch + 2 * v + jj
                                    nc.vector.tensor_scalar(
                                        ohp[:, jj, :],
                                        iota_t[:],
                                        labf[:, t : t + 1],
                                        None,
                                        op0=mybir.AluOpType.is_equal,
                                    )
                                ohs[u] = ohp
                        else:
                            for j in range(batch):
                                t = b * batch + j
                                oh_t = oh_pool.tile(
                                    [128, C_PAD], bf16, tag=f"oh{t}",
                                    name=f"oh{t}",
                                )
                                nc.vector.tensor_scalar(
                                    oh_t[:],
                                    iota_t[:],
                                    labf[:, t : t + 1],
                                    None,
                                    op0=mybir.AluOpType.is_equal,
                                )
                                ohs[t] = oh_t
                    if mm == "fp8":
                        for v in range(batch // 2):
                            u = b * (batch // 2) + v
                            for c in range(N_CHUNKS):
                                nc.tensor.matmul(
                                    accs[c][:],
                                    ohs[u][:, :, bass.ts(c, 128)],
                                    xsb[:, 2 * v : 2 * v + 2, :],
                                    perf_mode=mybir.MatmulPerfMode.DoubleRow,
                                    start=(u == 0),
                                    stop=(u == last_u),
                                )
                                if u == last_u:
                                    # drain as soon as this chunk retires so
                                    # the next pass reuses the bank stall-free;
                                    # alternate ACT/DVE so drains run 2-wide
                                    if c % 2 == 0:
                                        nc.scalar.copy(qstage[:, c, :], accs[c][:])
                                    else:
                                        nc.vector.tensor_copy(
                                            qstage[:, c, :], accs[c][:]
                                        )
                    else:
                        for j in range(batch):
                            t = b * batch + j
                            for c in range(N_CHUNKS):
                                nc.tensor.matmul(
                                    accs[c][:],
                                    ohs[t][:, bass.ts(c, 128)],
                                    xsb[:, j, :],
                                    start=(t == 0),
                                    stop=(t == last_u),
                                )
                                if t == last_u:
                                    if c % 2 == 0:
                                        nc.scalar.copy(qstage[:, c, :], accs[c][:])
                                    else:
                                        nc.vector.tensor_copy(
                                            qstage[:, c, :], accs[c][:]
                                        )

            issue_rs(3)

            # ---- tail: EMA + normalize + masked loss on this core's classes.
            # Everything below except the last quarter's chain overlaps the
            # final ReduceScatter. msums3 rides the idle SP queue so the
            # in-order ACT queue never blocks behind the RS3 wait.
            nc.scalar.dma_start(msums[:, bass.ts(1, Q)], rs_q[1][:, 0:Q])
            nc.scalar.dma_start(msums[:, bass.ts(2, Q)], rs_q[2][:, 0:Q])

            # ci/cs must beat msums3 onto the SP queue: msums3 blocks on the
            # final ReduceScatter, and the quarter-0..2 tail work needs ci/cs
            # to overlap that collective
            ci_sb = const_pool.tile([128, FEA], f32, tag="ci")
            nc.sync.dma_start(ci_sb[:], ci_d[:, :])
            cs_sb = const_pool.tile([128, FEA], f32, tag="cs")
            nc.sync.dma_start(cs_sb[:], cs_d[:, :])
            nc.sync.dma_start(msums[:, bass.ts(3, Q)], rs_q[3][:, 0:Q])

            # S3 = sum(cs^2) per class; independent of the collectives
            s3tmp = const_pool.tile([128, FEA], f32, tag="tailC")
            s3 = const_pool.tile([128, 1], f32, tag="s3")
            nc.scalar.activation(s3tmp[:], cs_sb[:], Sq, accum_out=s3[:])
            s3p1 = const_pool.tile([128, 1], f32, tag="s3p1")
            nc.vector.tensor_scalar(
                s3p1[:], s3[:], 1.0, None, op0=mybir.AluOpType.add
            )

            mcnt = const_pool.tile([128, 1], f32, tag="mcnt")
            nc.vector.tensor_copy(mcnt[:], mcntb[:])
            cnt1 = const_pool.tile([128, 1], f32, tag="cnt1")
            nc.vector.tensor_scalar_max(cnt1[:], mcnt[:], 1.0)
            rec = const_pool.tile([128, 1], f32, tag="rec")
            nc.vector.reciprocal(rec[:], cnt1[:])
            pres = const_pool.tile([128, 1], f32, tag="pres")
            nc.vector.tensor_scalar_min(pres[:], mcnt[:], 1.0)

            s1p = [const_pool.tile([128, 1], f32, tag=f"s1p{q}", name=f"s1p{q}")
                   for q in range(4)]
            s12p = [const_pool.tile([128, 1], f32, tag=f"s12p{q}", name=f"s12p{q}")
                    for q in range(4)]
            for q in range(4):
                qc = bass.ts(q, Q)
                # mean*(1-momentum) = sums * (1/count) * 0.1
                msc = const_pool.tile([128, Q], f32, tag="tailA")
                nc.vector.tensor_scalar(
                    msc[:],
                    msums[:, qc],
                    rec[:],
                    1.0 - MOMENTUM,
                    op0=mybir.AluOpType.mult,
                    op1=mybir.AluOpType.mult,
                )
                # upd = ci*momentum + mean*(1-momentum)
                upd = const_pool.tile([128, Q], f32, tag="tailB")
                nc.vector.scalar_tensor_tensor(
                    upd[:],
                    in0=ci_sb[:, qc],
                    scalar=MOMENTUM,
                    in1=msc[:],
                    op0=mybir.AluOpType.mult,
                    op1=mybir.AluOpType.add,
                )
                sqt = const_pool.tile([128, Q], f32, tag="tailC")
                nc.scalar.activation(sqt[:], upd[:], Sq, accum_out=s1p[q][:])
                ucs = const_pool.tile([128, Q], f32, tag="tailA")
                nc.vector.tensor_tensor(
                    ucs[:], upd[:], cs_sb[:, qc], op=mybir.AluOpType.add
                )
                sqt2 = const_pool.tile([128, Q], f32, tag="tailB")
                nc.scalar.activation(sqt2[:], ucs[:], Sq, accum_out=s12p[q][:])

            s1a = const_pool.tile([128, 1], f32, tag="s1a")
            nc.vector.tensor_tensor(s1a[:], s1p[0][:], s1p[1][:],
                                    op=mybir.AluOpType.add)
            s1b = const_pool.tile([128, 1], f32, tag="s1b")
            nc.vector.tensor_tensor(s1b[:], s1p[2][:], s1p[3][:],
                                    op=mybir.AluOpType.add)
            s1 = const_pool.tile([128, 1], f32, tag="s1")
            nc.vector.tensor_tensor(s1[:], s1a[:], s1b[:],
                                    op=mybir.AluOpType.add)
            s12a = const_pool.tile([128, 1], f32, tag="s12a")
            nc.vector.tensor_tensor(s12a[:], s12p[0][:], s12p[1][:],
                                    op=mybir.AluOpType.add)
            s12b = const_pool.tile([128, 1], f32, tag="s12b")
            nc.vector.tensor_tensor(s12b[:], s12p[2][:], s12p[3][:],
                                    op=mybir.AluOpType.add)
            s12 = const_pool.tile([128, 1], f32, tag="s12")
            nc.vector.tensor_tensor(s12[:], s12a[:], s12b[:],
                                    op=mybir.AluOpType.add)

            # per_cls = (1 + S3) - (S12 - S1 - S3) / sqrt(S1)
            s1g = const_pool.tile([128, 1], f32, tag="s1g")
            nc.vector.tensor_scalar_max(s1g[:], s1[:], 1e-30)
            s1r = const_pool.tile([128, 1], f32, tag="s1r")
            nc.vector.reciprocal(s1r[:], s1g[:])
            rsq = const_pool.tile([128, 1], f32, tag="rsq")
            nc.scalar.activation(
                rsq[:], s1r[:], mybir.ActivationFunctionType.Sqrt
            )
            t0 = const_pool.tile([128, 1], f32, tag="t0")
            nc.vector.tensor_tensor(t0[:], s12[:], s1[:],
                                    op=mybir.AluOpType.subtract)
            t1 = const_pool.tile([128, 1], f32, tag="t1")
            nc.vector.tensor_tensor(t1[:], t0[:], s3[:],
                                    op=mybir.AluOpType.subtract)
            t2 = const_pool.tile([128, 1], f32, tag="t2")
            nc.vector.tensor_tensor(t2[:], t1[:], rsq[:],
                                    op=mybir.AluOpType.mult)
            per = const_pool.tile([128, 1], f32, tag="per")
            nc.vector.tensor_tensor(per[:], s3p1[:], t2[:],
                                    op=mybir.AluOpType.subtract)
            stack = const_pool.tile([128, 2], f32, tag="stack")
            nc.vector.tensor_tensor(
                stack[:, 0:1], per[:], pres[:], op=mybir.AluOpType.mult
            )
            nc.vector.tensor_copy(stack[:, 1:2], pres[:])
            nc.sync.dma_start(out_d[:, :], stack[:])

    nc.compile()
    return nc


def make_in_maps(x, center_img, center_skt, l, rows_per_core=ROWS_PER_CORE):
    """Shard full inputs into per-core input maps (x slices are views)."""
    n = x.shape[0] // NUM_CROPS
    x = np.ascontiguousarray(x, dtype=np.float32)
    l = np.ascontiguousarray(l).astype(np.int32)
    ci_pad = np.zeros((C_PAD, FEA), np.float32)
    ci_pad[: center_img.shape[0]] = center_img
    cs_pad = np.zeros((C_PAD, FEA), np.float32)
    cs_pad[: center_skt.shape[0]] = center_skt
    in_maps = []
    for k in range(N_CORES):
        r0 = k * rows_per_core
        r1 = r0 + rows_per_core
        in_maps.append(
            {
                "x0": x[r0:r1],
                "x1": x[n + r0 : n + r1],
                "labels": l[r0:r1],
                "ci": ci_pad[k * 128 : (k + 1) * 128],
                "cs": cs_pad[k * 128 : (k + 1) * 128],
            }
        )
    return in_maps


def reduce_outputs(res):
    """Host-side unshard: combine per-core [128, 2] partials into the loss."""
    parts = np.stack([np.asarray(res[c]["loss"], np.float64) for c in range(N_CORES)])
    loss_sum = parts[:, :, 0].sum()
    n_present = parts[:, :, 1].sum()
    return np.float32(loss_sum / n_present)


_CACHED_NC = None


def _get_nc():
    global _CACHED_NC
    if _CACHED_NC is None:
        _CACHED_NC = build_program()
    return _CACHED_NC


def kernel(x, center_img, center_skt, l):
    nc = _get_nc()
    in_maps = make_in_maps(x, center_img, center_skt, l)
    res = bass_utils.run_bass_kernel_spmd(nc, in_maps, core_ids=list(range(N_CORES)))
    return reduce_outputs(res.results).reshape(()).astype(np.float32)
